# revision 6
# baseline (speedup 1.0000x reference)
"""Trainium2 Bass kernel for the MinGRU language-model layer stack.

kernel(**inputs) takes the unsharded inputs of reference.setup_inputs()
and returns (loss, logits, next_hidden) like reference.reference().

Strategy (8 NeuronCores, one SPMD program):
  - The dominant cost is logits = h_norm @ W_logits ([4096,1024] x
    [1024,50257], ~422 of ~507 GFLOP). The vocab dim is sharded 8 ways;
    each core streams its W_logits shard from HBM once.
  - The trunk (embedding gather, RMSNorm, minGRU, FFN) runs on every
    core; the sequential minGRU scan is chained across token chunks via
    tensor_tensor_scan initial-value chaining.
  - minGRU runs in linear space: h_t = a_t*h_{t-1} + b_t with
    a = sigmoid(-gate), b = sigmoid(gate)*g(hidden) — mathematically
    identical to the reference's log-space Heinsen scan, and stable here
    since a in (0,1), b > 0. One VectorE tensor_tensor_scan per
    128-channel tile runs the recurrence along the token axis.
  - Matmuls run in float32r (fp32 rounded to 11 mantissa bits, 4x the
    fp32 PE rate); PSUM accumulates in full fp32.
  - Per-row sum(exp(logit)) over each vocab shard is computed on device
    (ScalarE Exp + accumulate). The host does only the cross-shard LSE
    combine, the label gather from the returned logits, the scalar mean,
    and output unsharding. |logits| <= ~10 here, so the plain sum-exp is
    safe without max subtraction.

SBUF/PSUM budgets (per partition, CT=512): trunk ~183KB SBUF and exactly
8 PSUM banks (tr,gr,ph,pg + pf x2 + pf2a,pf2b); logits phase ~170KB SBUF
(h_norm^T resident as the stationary matmul operand, W_logits streamed).
"""

import os
import sys
import math

for _p in ("/opt/trn_rl_repo", "/root/.axon_site/_ro/trn_rl_repo"):
    if os.path.isdir(_p) and _p not in sys.path:
        sys.path.insert(0, _p)

import numpy as np

import concourse.bass as bass
import concourse.tile as tile
from concourse import bacc, mybir
from concourse.bass_utils import run_bass_kernel_spmd
from concourse.masks import make_identity

FP32 = mybir.dt.float32
F32R = mybir.dt.float32r
I32 = mybir.dt.int32
AF = mybir.ActivationFunctionType
ALU = mybir.AluOpType
P = 128

# swappable for CoreSim (which lacks a Gelu model); HW uses the exact-erf Gelu LUT
GELU_FUNC = AF.Gelu


class Cfg:
    def __init__(self, D=1024, V=50257, B=2, S=2048, ncores=8, chunk=512):
        self.D = D
        self.V = V
        self.B = B
        self.S = S
        self.ncores = ncores
        self.CT = chunk
        self.NT = B * S
        self.KD = D // P
        self.FF = 4 * D
        self.KF = self.FF // P
        self.NCH = self.NT // chunk
        self.TBC = chunk // P
        self.NTB = self.NT // P
        # fp32r matmuls need the moving free dim >=256 (full rate) and the
        # ISA rejects ragged widths; round the shard up to a multiple of 256.
        self.VSH = -(-((V + ncores - 1) // ncores) // 256) * 256
        self.pad_last = self.VSH * ncores - V
        self.VBW = []
        r = self.VSH
        while r > 0:
            w = min(512, r)
            self.VBW.append(w)
            r -= w


def build_program(cfg: Cfg):
    c = cfg
    nc = bacc.Bacc("TRN2", target_bir_lowering=False, debug=False,
                   num_devices=c.ncores)

    TOK = nc.dram_tensor("tok", [c.NTB, P], I32, kind="ExternalInput")
    EMB = nc.dram_tensor("emb", [c.V, c.D], FP32, kind="ExternalInput")
    G1 = nc.dram_tensor("g1", [c.KD, P], FP32, kind="ExternalInput")
    G2 = nc.dram_tensor("g2", [c.KD, P], FP32, kind="ExternalInput")
    B1 = nc.dram_tensor("b1", [c.KF, P], FP32, kind="ExternalInput")
    B2 = nc.dram_tensor("b2", [c.KD, P], FP32, kind="ExternalInput")
    WHG = nc.dram_tensor("whg", [c.D, 2 * c.D], F32R, kind="ExternalInput")
    W1 = nc.dram_tensor("w1", [c.D, c.FF], F32R, kind="ExternalInput")
    W2 = nc.dram_tensor("w2", [c.FF, c.D], F32R, kind="ExternalInput")
    WLOG = nc.dram_tensor("wlog", [c.D, c.VSH], F32R, kind="ExternalInput")

    LOGITS = nc.dram_tensor("logits_s", [c.NT, c.VSH], FP32,
                            kind="ExternalOutput")
    SUMEXP = nc.dram_tensor("sumexp", [P, c.NTB], FP32, kind="ExternalOutput")
    GLAST = nc.dram_tensor("gru_last", [c.B, c.D], FP32, kind="ExternalOutput")

    # gamma2-scaled normed hidden state, ^T layout [D, NT], staged in DRAM
    HN = nc.dram_tensor("hN_stage", [c.D, c.NT], F32R)

    SQD = float(math.sqrt(c.D))

    with tile.TileContext(nc) as tc:
        with tc.tile_pool(name="persist", bufs=1) as pp:
            ident = pp.tile([P, P], FP32)
            make_identity(nc, ident[:])

            tok_sb = pp.tile([P, c.NTB], I32)
            for tb in range(c.NTB):
                nc.sync.dma_start(tok_sb[:, tb:tb + 1], TOK[tb:tb + 1, :])

            g1p = pp.tile([P, c.KD], FP32)
            g2p = pp.tile([P, c.KD], FP32)
            b2_sb = pp.tile([P, c.KD], FP32)
            b1_sb = pp.tile([P, c.KF], FP32)
            for k in range(c.KD):
                nc.sync.dma_start(g1p[:, k:k + 1], G1[k:k + 1, :])
                nc.sync.dma_start(g2p[:, k:k + 1], G2[k:k + 1, :])
                nc.sync.dma_start(b2_sb[:, k:k + 1], B2[k:k + 1, :])
            for m in range(c.KF):
                nc.sync.dma_start(b1_sb[:, m:m + 1], B1[m:m + 1, :])
            nc.vector.tensor_scalar(g1p[:], g1p[:], 1.0, None, op0=ALU.add)
            nc.vector.tensor_scalar(g2p[:], g2p[:], 1.0, None, op0=ALU.add)

            ftok = pp.tile([P, c.NTB], FP32)
            se_acc = pp.tile([P, c.NTB], FP32)
            nc.vector.memset(se_acc[:], 0.0)
            chain = pp.tile([P, c.KD], FP32)

            # =================== trunk ===================
            with (
                tc.tile_pool(name="trunk", bufs=1) as tp,
                tc.tile_pool(name="gpool", bufs=2) as gp,
                tc.tile_pool(name="tw1", bufs=1) as tw1,
                tc.tile_pool(name="tw2", bufs=2) as tw2,
                tc.tile_pool(name="psA", bufs=1, space="PSUM") as psA,
                tc.tile_pool(name="psB", bufs=2, space="PSUM") as psB,
            ):
                for ch in range(c.NCH):
                    batch_start = (ch % (c.NCH // c.B) == 0)
                    batch_end = ((ch + 1) % (c.NCH // c.B) == 0)
                    bidx = ch // (c.NCH // c.B)

                    # ---- gather + rmsnorm1 + transpose (-> hTr, f32r) ----
                    hTr = [tp.tile([P, c.CT], F32R, tag=f"hTr{k}", name=f"hTr{k}")
                           for k in range(c.KD)]
                    for tb in range(c.TBC):
                        gtb = ch * c.TBC + tb
                        h0 = tp.tile([P, c.D], FP32, tag="h0")
                        nc.gpsimd.indirect_dma_start(
                            out=h0[:], out_offset=None,
                            in_=EMB[:],
                            in_offset=bass.IndirectOffsetOnAxis(
                                ap=tok_sb[:, gtb:gtb + 1], axis=0),
                        )
                        h0n = tp.tile([P, c.D], FP32, tag="h0n")
                        ss = tp.tile([P, 1], FP32, tag="ss")
                        nc.scalar.activation(h0n[:], h0[:], AF.Square,
                                             accum_out=ss[:])
                        nc.scalar.activation(ss[:], ss[:], AF.Sqrt)
                        nc.vector.tensor_scalar_max(ss[:], ss[:], 1e-12)
                        rn = tp.tile([P, 1], FP32, tag="rn")
                        nc.vector.reciprocal(rn[:], ss[:])
                        nc.vector.tensor_scalar(
                            h0n[:], h0[:], rn[:, 0:1], SQD,
                            op0=ALU.mult, op1=ALU.mult)
                        for k in range(c.KD):
                            ptr = psA.tile([P, P], FP32, tag="tr")
                            nc.tensor.transpose(
                                ptr[:], h0n[:, k * P:(k + 1) * P], ident[:])
                            nc.vector.tensor_scalar(
                                hTr[k][:, tb * P:(tb + 1) * P], ptr[:],
                                g1p[:, k:k + 1], None, op0=ALU.mult)

                    # ---- minGRU ----
                    x1r = [tp.tile([P, c.CT], F32R, tag=f"x1r{k}", name=f"x1r{k}")
                           for k in range(c.KD)]
                    for i in range(c.KD):
                        whs = [tw1.tile([P, P], F32R, tag=f"whs{k}", name=f"whs{k}")
                               for k in range(c.KD)]
                        wgs = [tw1.tile([P, P], F32R, tag=f"wgs{k}", name=f"wgs{k}")
                               for k in range(c.KD)]
                        for k in range(c.KD):
                            nc.sync.dma_start(
                                whs[k][:],
                                WHG[k * P:(k + 1) * P, i * P:(i + 1) * P])
                            nc.sync.dma_start(
                                wgs[k][:],
                                WHG[k * P:(k + 1) * P,
                                    c.D + i * P:c.D + (i + 1) * P])
                        ph = psA.tile([P, c.CT], FP32, tag="ph")
                        pg = psA.tile([P, c.CT], FP32, tag="pg")
                        for k in range(c.KD):
                            nc.tensor.matmul(ph[:], whs[k][:], hTr[k][:],
                                             start=(k == 0),
                                             stop=(k == c.KD - 1))
                        for k in range(c.KD):
                            nc.tensor.matmul(pg[:], wgs[k][:], hTr[k][:],
                                             start=(k == 0),
                                             stop=(k == c.KD - 1))
                        a_t = tp.tile([P, c.CT], FP32, tag="a_t")
                        nc.scalar.activation(a_t[:], pg[:], AF.Sigmoid,
                                             scale=-1.0)
                        s_t = tp.tile([P, c.CT], FP32, tag="s_t")
                        nc.scalar.activation(s_t[:], ph[:], AF.Sigmoid)
                        # g = s + (h>=0)*(h+0.5-s) ; b = g - a*g  (z = 1-a)
                        g_t = tp.tile([P, c.CT], FP32, tag="g_t")
                        nc.vector.tensor_scalar(g_t[:], ph[:], 0.5, None,
                                                op0=ALU.add)
                        nc.vector.tensor_tensor(g_t[:], g_t[:], s_t[:],
                                                op=ALU.subtract)
                        m_t = tp.tile([P, c.CT], FP32, tag="m_t")
                        nc.vector.tensor_scalar(m_t[:], ph[:], 0.0, None,
                                                op0=ALU.is_ge)
                        nc.vector.tensor_tensor(g_t[:], g_t[:], m_t[:],
                                                op=ALU.mult)
                        nc.vector.tensor_tensor(g_t[:], g_t[:], s_t[:],
                                                op=ALU.add)
                        nc.vector.tensor_tensor(m_t[:], a_t[:], g_t[:],
                                                op=ALU.mult)
                        nc.vector.tensor_tensor(s_t[:], g_t[:], m_t[:],
                                                op=ALU.subtract)
                        gru = gp.tile([P, c.CT], FP32, tag="gru")
                        init = 0.0 if batch_start else chain[:, i:i + 1]
                        nc.vector.tensor_tensor_scan(
                            gru[:], a_t[:], s_t[:], init,
                            op0=ALU.mult, op1=ALU.add)
                        nc.vector.tensor_copy(chain[:, i:i + 1],
                                              gru[:, c.CT - 1:c.CT])
                        nc.vector.tensor_tensor(x1r[i][:], gru[:],
                                                hTr[i][:], op=ALU.add)
                        if batch_end:
                            nc.sync.dma_start(
                                GLAST[bidx:bidx + 1, i * P:(i + 1) * P],
                                chain[:, i:i + 1])

                    # ---- FFN1: h1 = gelu(x1 @ W1 + b1) ----
                    h1r = [tp.tile([P, c.CT], F32R, tag=f"h1r{m}", name=f"h1r{m}")
                           for m in range(c.KF)]
                    for m in range(c.KF):
                        if m % 4 == 0:
                            w1s = [tw1.tile([P, 4 * P], F32R, tag=f"w1s{k}", name=f"w1s{k}")
                                   for k in range(c.KD)]
                            for k in range(c.KD):
                                nc.sync.dma_start(
                                    w1s[k][:], W1[k * P:(k + 1) * P,
                                                  m * P:(m + 4) * P])
                        pf = psB.tile([P, c.CT], FP32, tag="pf")
                        off = (m % 4) * P
                        for k in range(c.KD):
                            nc.tensor.matmul(
                                pf[:], w1s[k][:, off:off + P], x1r[k][:],
                                start=(k == 0), stop=(k == c.KD - 1))
                        nc.scalar.activation(h1r[m][:], pf[:], GELU_FUNC,
                                             bias=b1_sb[:, m:m + 1])

                    # ---- FFN2 + residual; x2g staged to DRAM ----
                    x2r = [tp.tile([P, c.CT], F32R, tag=f"x2r{k}", name=f"x2r{k}")
                           for k in range(c.KD)]
                    for mp in range(c.KD // 2):  # m2 pairs
                        pa = psA.tile([P, c.CT], FP32, tag="pf2a")
                        pb = psA.tile([P, c.CT], FP32, tag="pf2b")
                        for k2 in range(c.KF):
                            w2s = tw2.tile([P, 2 * P], F32R,
                                           tag=f"w2s{k2 % 8}")
                            nc.sync.dma_start(
                                w2s[:], W2[k2 * P:(k2 + 1) * P,
                                           mp * 2 * P:(mp + 1) * 2 * P])
                            nc.tensor.matmul(pa[:], w2s[:, 0:P], h1r[k2][:],
                                             start=(k2 == 0),
                                             stop=(k2 == c.KF - 1))
                            nc.tensor.matmul(pb[:], w2s[:, P:2 * P],
                                             h1r[k2][:],
                                             start=(k2 == 0),
                                             stop=(k2 == c.KF - 1))
                        for m2, pf2 in ((2 * mp, pa), (2 * mp + 1, pb)):
                            nc.vector.scalar_tensor_tensor(
                                x2r[m2][:], pf2[:], b2_sb[:, m2:m2 + 1],
                                x1r[m2][:], op0=ALU.add, op1=ALU.add)
                            x2g = tp.tile([P, c.CT], F32R, tag="x2g")
                            nc.vector.tensor_scalar(
                                x2g[:], x2r[m2][:], g2p[:, m2:m2 + 1], None,
                                op0=ALU.mult)
                            nc.sync.dma_start(
                                HN[m2 * P:(m2 + 1) * P,
                                   ch * c.CT:(ch + 1) * c.CT], x2g[:])

                    # ---- rmsnorm2 stats via gram diagonal ----
                    for tb in range(c.TBC):
                        gtb = ch * c.TBC + tb
                        pgr = psA.tile([P, P], FP32, tag="gr")
                        for k in range(c.KD):
                            sl = x2r[k][:, tb * P:(tb + 1) * P]
                            nc.tensor.matmul(pgr[:], sl, sl,
                                             start=(k == 0),
                                             stop=(k == c.KD - 1))
                        dg = tp.tile([P, P], FP32, tag="dg")
                        nc.vector.tensor_tensor(dg[:], pgr[:], ident[:],
                                                op=ALU.mult)
                        ss2 = tp.tile([P, 1], FP32, tag="ss2")
                        nc.vector.tensor_reduce(ss2[:], dg[:],
                                                axis=mybir.AxisListType.X,
                                                op=ALU.add)
                        nc.scalar.activation(ss2[:], ss2[:], AF.Sqrt)
                        nc.vector.tensor_scalar_max(ss2[:], ss2[:], 1e-12)
                        nc.vector.reciprocal(ftok[:, gtb:gtb + 1], ss2[:])

            # =================== logits ===================
            with (
                tc.tile_pool(name="lg", bufs=1) as lp,
                tc.tile_pool(name="lgw", bufs=2) as lw,
                tc.tile_pool(name="lgm", bufs=3) as lm,
                tc.tile_pool(name="psL", bufs=3, space="PSUM") as psL,
            ):
                hNr = [lp.tile([P, c.NT], F32R, tag=f"hNr{k}", name=f"hNr{k}")
                       for k in range(c.KD)]
                for k in range(c.KD):
                    nc.sync.dma_start(hNr[k][:], HN[k * P:(k + 1) * P, :])
                v0 = 0
                for vb, w in enumerate(c.VBW):
                    wv = [lw.tile([P, w], F32R, tag=f"wv{k}", name=f"wv{k}")
                          for k in range(c.KD)]
                    for k in range(c.KD):
                        nc.sync.dma_start(
                            wv[k][:], WLOG[k * P:(k + 1) * P, v0:v0 + w])
                    for tb in range(c.NTB):
                        pl = psL.tile([P, w], FP32, tag="pl")
                        for k in range(c.KD):
                            nc.tensor.matmul(
                                pl[:], hNr[k][:, tb * P:(tb + 1) * P],
                                wv[k][:],
                                start=(k == 0), stop=(k == c.KD - 1))
                        lg_sb = lm.tile([P, w], FP32, tag="lg_sb")
                        nc.vector.tensor_scalar(
                            lg_sb[:], pl[:], ftok[:, tb:tb + 1], SQD,
                            op0=ALU.mult, op1=ALU.mult)
                        nc.sync.dma_start(
                            LOGITS[tb * P:(tb + 1) * P, v0:v0 + w], lg_sb[:])
                        scr = lm.tile([P, w], FP32, tag="scr")
                        se = lm.tile([P, 1], FP32, tag="se")
                        nc.scalar.activation(scr[:], lg_sb[:], AF.Exp,
                                             accum_out=se[:])
                        nc.vector.tensor_tensor(
                            se_acc[:, tb:tb + 1], se_acc[:, tb:tb + 1],
                            se[:], op=ALU.add)
                    v0 += w
                nc.sync.dma_start(SUMEXP[:], se_acc[:])

    nc.compile()
    return nc


_CACHE = {}


def _get_program(cfg: Cfg):
    key = (cfg.D, cfg.V, cfg.B, cfg.S, cfg.ncores, cfg.CT)
    if key not in _CACHE:
        _CACHE[key] = build_program(cfg)
    return _CACHE[key]


def make_in_maps(cfg, x, emb, gamma1, W_hg, W1, b1, W2, b2, gamma2, W_logits):
    c = cfg
    x = np.asarray(x)
    tok = np.ascontiguousarray(
        np.asarray(x[:, :-1]).reshape(-1).astype(np.int32)
    ).reshape(c.NTB, P)
    common = {
        "tok": tok,
        "emb": np.ascontiguousarray(np.asarray(emb, np.float32)),
        "g1": np.asarray(gamma1, np.float32).reshape(c.KD, P),
        "g2": np.asarray(gamma2, np.float32).reshape(c.KD, P),
        "b1": np.asarray(b1, np.float32).reshape(c.KF, P),
        "b2": np.asarray(b2, np.float32).reshape(c.KD, P),
        "whg": np.ascontiguousarray(np.asarray(W_hg, np.float32)),
        "w1": np.ascontiguousarray(np.asarray(W1, np.float32)),
        "w2": np.ascontiguousarray(np.asarray(W2, np.float32)),
    }
    wl = np.asarray(W_logits, np.float32)
    in_maps = []
    for r in range(c.ncores):
        sl = wl[:, r * c.VSH:(r + 1) * c.VSH]
        if sl.shape[1] < c.VSH:
            sl = np.pad(sl, ((0, 0), (0, c.VSH - sl.shape[1])))
        m = dict(common)
        m["wlog"] = np.ascontiguousarray(sl)
        in_maps.append(m)
    return in_maps


def assemble(cfg, x, results):
    c = cfg
    x = np.asarray(x)
    labels = np.asarray(x[:, 1:]).reshape(-1).astype(np.int64)

    logits = np.empty((c.NT, c.V), np.float32)
    se_total = np.zeros(c.NT, np.float64)
    for r in range(c.ncores):
        w = min(c.VSH, c.V - r * c.VSH)
        logits[:, r * c.VSH:r * c.VSH + w] = results[r]["logits_s"][:, :w]
        se = results[r]["sumexp"].astype(np.float64)  # [P, NTB]
        se = se.T.reshape(-1)                         # token = tb*P + p
        if w < c.VSH:
            se = se - (c.VSH - w)  # zero-padded cols contribute exp(0)=1
        se_total += se

    lse = np.log(se_total)
    label_logit = logits[np.arange(c.NT), labels].astype(np.float64)
    loss = np.float32(np.mean(lse - label_logit))

    next_hidden = results[0]["gru_last"].reshape(
        c.B, 1, c.D).astype(np.float32)
    return loss, logits.reshape(c.B, c.S, c.V), next_hidden


def kernel(x, emb, gamma1, W_hg, W1, b1, W2, b2, gamma2, W_logits):
    cfg = Cfg()
    nc = _get_program(cfg)
    in_maps = make_in_maps(cfg, x, emb, gamma1, W_hg, W1, b1, W2, b2,
                           gamma2, W_logits)
    res = run_bass_kernel_spmd(nc, in_maps, list(range(cfg.ncores)))
    return assemble(cfg, x, res.results)


# revision 9
# speedup vs baseline: 1.9083x; 1.9083x over previous
"""Trainium2 Bass kernel for the MinGRU language-model layer stack.

kernel(**inputs) takes the unsharded inputs of reference.setup_inputs()
and returns (loss, logits, next_hidden) like reference.reference().

Strategy (8 NeuronCores, one SPMD program):
  - The dominant cost is logits = h_norm @ W_logits ([4096,1024] x
    [1024,50257], ~422 of ~507 GFLOP). The vocab dim is sharded 8 ways;
    each core streams its W_logits shard from HBM once.
  - The trunk (embedding gather, RMSNorm, minGRU, FFN) runs on every
    core; the sequential minGRU scan is chained across token chunks via
    tensor_tensor_scan initial-value chaining.
  - minGRU runs in linear space: h_t = a_t*h_{t-1} + b_t with
    a = sigmoid(-gate), b = sigmoid(gate)*g(hidden) — mathematically
    identical to the reference's log-space Heinsen scan, and stable here
    since a in (0,1), b > 0. One VectorE tensor_tensor_scan per
    128-channel tile runs the recurrence along the token axis.
  - Matmuls run in float32r (fp32 rounded to 11 mantissa bits, 4x the
    fp32 PE rate); PSUM accumulates in full fp32.
  - Per-row sum(exp(logit)) over each vocab shard is computed on device
    (ScalarE Exp + accumulate). The host does only the cross-shard LSE
    combine, the label gather from the returned logits, the scalar mean,
    and output unsharding. |logits| <= ~10 here, so the plain sum-exp is
    safe without max subtraction.

SBUF/PSUM budgets (per partition, CT=512): trunk ~183KB SBUF and exactly
8 PSUM banks (tr,gr,ph,pg + pf x2 + pf2a,pf2b); logits phase ~170KB SBUF
(h_norm^T resident as the stationary matmul operand, W_logits streamed).
"""

import os
import sys
import math

for _p in ("/opt/trn_rl_repo", "/root/.axon_site/_ro/trn_rl_repo"):
    if os.path.isdir(_p) and _p not in sys.path:
        sys.path.insert(0, _p)

import numpy as np

import concourse.bass as bass
import concourse.tile as tile
from concourse import bacc, mybir
from concourse.bass_utils import run_bass_kernel_spmd
from concourse.masks import make_identity

FP32 = mybir.dt.float32
F32R = mybir.dt.float32r
I32 = mybir.dt.int32
AF = mybir.ActivationFunctionType
ALU = mybir.AluOpType
P = 128

# swappable for CoreSim (which lacks a Gelu model); HW uses the exact-erf Gelu LUT
GELU_FUNC = AF.Gelu


class Cfg:
    def __init__(self, D=1024, V=50257, B=2, S=2048, ncores=8, chunk=512):
        self.D = D
        self.V = V
        self.B = B
        self.S = S
        self.ncores = ncores
        self.CT = chunk
        self.NT = B * S
        self.KD = D // P
        self.FF = 4 * D
        self.KF = self.FF // P
        self.NCH = self.NT // chunk
        self.TBC = chunk // P
        self.NTB = self.NT // P
        # fp32r matmuls need the moving free dim >=256 (full rate) and the
        # ISA rejects ragged widths; round the shard up to a multiple of 256.
        self.VSH = -(-((V + ncores - 1) // ncores) // 256) * 256
        self.pad_last = self.VSH * ncores - V
        self.G = ncores // B  # cores per batch (seq-parallel mode)
        self.VBW = []
        r = self.VSH
        while r > 0:
            w = min(512, r)
            self.VBW.append(w)
            r -= w


def build_program(cfg: Cfg):
    c = cfg
    nc = bacc.Bacc("TRN2", target_bir_lowering=False, debug=False,
                   num_devices=c.ncores)

    TOK = nc.dram_tensor("tok", [c.NTB, P], I32, kind="ExternalInput")
    EMB = nc.dram_tensor("emb", [c.V, c.D], FP32, kind="ExternalInput")
    G1 = nc.dram_tensor("g1", [c.KD, P], FP32, kind="ExternalInput")
    G2 = nc.dram_tensor("g2", [c.KD, P], FP32, kind="ExternalInput")
    B1 = nc.dram_tensor("b1", [c.KF, P], FP32, kind="ExternalInput")
    B2 = nc.dram_tensor("b2", [c.KD, P], FP32, kind="ExternalInput")
    WHG = nc.dram_tensor("whg", [c.D, 2 * c.D], F32R, kind="ExternalInput")
    W1 = nc.dram_tensor("w1", [c.D, c.FF], F32R, kind="ExternalInput")
    W2 = nc.dram_tensor("w2", [c.FF, c.D], F32R, kind="ExternalInput")
    WLOG = nc.dram_tensor("wlog", [c.D, c.VSH], F32R, kind="ExternalInput")

    LOGITS = nc.dram_tensor("logits_s", [c.NT, c.VSH], FP32,
                            kind="ExternalOutput")
    SUMEXP = nc.dram_tensor("sumexp", [P, c.NTB], FP32, kind="ExternalOutput")
    GLAST = nc.dram_tensor("gru_last", [c.B, c.D], FP32, kind="ExternalOutput")

    # gamma2-scaled normed hidden state, ^T layout [D, NT], staged in DRAM
    HN = nc.dram_tensor("hN_stage", [c.D, c.NT], F32R)

    SQD = float(math.sqrt(c.D))

    with tile.TileContext(nc) as tc:
        with tc.tile_pool(name="persist", bufs=1) as pp:
            ident = pp.tile([P, P], FP32)
            make_identity(nc, ident[:])

            tok_sb = pp.tile([P, c.NTB], I32)
            for tb in range(c.NTB):
                nc.sync.dma_start(tok_sb[:, tb:tb + 1], TOK[tb:tb + 1, :])

            g1p = pp.tile([P, c.KD], FP32)
            g2p = pp.tile([P, c.KD], FP32)
            b2_sb = pp.tile([P, c.KD], FP32)
            b1_sb = pp.tile([P, c.KF], FP32)
            for k in range(c.KD):
                nc.sync.dma_start(g1p[:, k:k + 1], G1[k:k + 1, :])
                nc.sync.dma_start(g2p[:, k:k + 1], G2[k:k + 1, :])
                nc.sync.dma_start(b2_sb[:, k:k + 1], B2[k:k + 1, :])
            for m in range(c.KF):
                nc.sync.dma_start(b1_sb[:, m:m + 1], B1[m:m + 1, :])
            nc.vector.tensor_scalar(g1p[:], g1p[:], 1.0, None, op0=ALU.add)
            nc.vector.tensor_scalar(g2p[:], g2p[:], 1.0, None, op0=ALU.add)

            ftok = pp.tile([P, c.NTB], FP32)
            se_acc = pp.tile([P, c.NTB], FP32)
            nc.vector.memset(se_acc[:], 0.0)
            chain = pp.tile([P, c.KD], FP32)

            # =================== trunk ===================
            with (
                tc.tile_pool(name="trunk", bufs=1) as tp,
                tc.tile_pool(name="gpool", bufs=2) as gp,
                tc.tile_pool(name="tw1", bufs=1) as tw1,
                tc.tile_pool(name="tw2", bufs=2) as tw2,
                tc.tile_pool(name="psA", bufs=1, space="PSUM") as psA,
                tc.tile_pool(name="psB", bufs=2, space="PSUM") as psB,
            ):
                for ch in range(c.NCH):
                    batch_start = (ch % (c.NCH // c.B) == 0)
                    batch_end = ((ch + 1) % (c.NCH // c.B) == 0)
                    bidx = ch // (c.NCH // c.B)

                    # ---- gather + rmsnorm1 + transpose (-> hTr, f32r) ----
                    hTr = [tp.tile([P, c.CT], F32R, tag=f"hTr{k}", name=f"hTr{k}")
                           for k in range(c.KD)]
                    for tb in range(c.TBC):
                        gtb = ch * c.TBC + tb
                        h0 = tp.tile([P, c.D], FP32, tag="h0")
                        nc.gpsimd.indirect_dma_start(
                            out=h0[:], out_offset=None,
                            in_=EMB[:],
                            in_offset=bass.IndirectOffsetOnAxis(
                                ap=tok_sb[:, gtb:gtb + 1], axis=0),
                        )
                        h0n = tp.tile([P, c.D], FP32, tag="h0n")
                        ss = tp.tile([P, 1], FP32, tag="ss")
                        nc.scalar.activation(h0n[:], h0[:], AF.Square,
                                             accum_out=ss[:])
                        nc.scalar.activation(ss[:], ss[:], AF.Sqrt)
                        nc.vector.tensor_scalar_max(ss[:], ss[:], 1e-12)
                        rn = tp.tile([P, 1], FP32, tag="rn")
                        nc.vector.reciprocal(rn[:], ss[:])
                        nc.vector.tensor_scalar(
                            h0n[:], h0[:], rn[:, 0:1], SQD,
                            op0=ALU.mult, op1=ALU.mult)
                        for k in range(c.KD):
                            ptr = psA.tile([P, P], FP32, tag="tr")
                            nc.tensor.transpose(
                                ptr[:], h0n[:, k * P:(k + 1) * P], ident[:])
                            nc.vector.tensor_scalar(
                                hTr[k][:, tb * P:(tb + 1) * P], ptr[:],
                                g1p[:, k:k + 1], None, op0=ALU.mult)

                    # ---- minGRU ----
                    x1r = [tp.tile([P, c.CT], F32R, tag=f"x1r{k}", name=f"x1r{k}")
                           for k in range(c.KD)]
                    for i in range(c.KD):
                        whs = [tw1.tile([P, P], F32R, tag=f"whs{k}", name=f"whs{k}")
                               for k in range(c.KD)]
                        wgs = [tw1.tile([P, P], F32R, tag=f"wgs{k}", name=f"wgs{k}")
                               for k in range(c.KD)]
                        for k in range(c.KD):
                            nc.sync.dma_start(
                                whs[k][:],
                                WHG[k * P:(k + 1) * P, i * P:(i + 1) * P])
                            nc.sync.dma_start(
                                wgs[k][:],
                                WHG[k * P:(k + 1) * P,
                                    c.D + i * P:c.D + (i + 1) * P])
                        ph = psA.tile([P, c.CT], FP32, tag="ph")
                        pg = psA.tile([P, c.CT], FP32, tag="pg")
                        for k in range(c.KD):
                            nc.tensor.matmul(ph[:], whs[k][:], hTr[k][:],
                                             start=(k == 0),
                                             stop=(k == c.KD - 1))
                        for k in range(c.KD):
                            nc.tensor.matmul(pg[:], wgs[k][:], hTr[k][:],
                                             start=(k == 0),
                                             stop=(k == c.KD - 1))
                        a_t = tp.tile([P, c.CT], FP32, tag="a_t")
                        nc.scalar.activation(a_t[:], pg[:], AF.Sigmoid,
                                             scale=-1.0)
                        s_t = tp.tile([P, c.CT], FP32, tag="s_t")
                        nc.scalar.activation(s_t[:], ph[:], AF.Sigmoid)
                        # g = s + (h>=0)*(h+0.5-s) ; b = g - a*g  (z = 1-a)
                        g_t = tp.tile([P, c.CT], FP32, tag="g_t")
                        nc.vector.tensor_scalar(g_t[:], ph[:], 0.5, None,
                                                op0=ALU.add)
                        nc.vector.tensor_tensor(g_t[:], g_t[:], s_t[:],
                                                op=ALU.subtract)
                        m_t = tp.tile([P, c.CT], FP32, tag="m_t")
                        nc.vector.tensor_scalar(m_t[:], ph[:], 0.0, None,
                                                op0=ALU.is_ge)
                        nc.vector.tensor_tensor(g_t[:], g_t[:], m_t[:],
                                                op=ALU.mult)
                        nc.vector.tensor_tensor(g_t[:], g_t[:], s_t[:],
                                                op=ALU.add)
                        nc.vector.tensor_tensor(m_t[:], a_t[:], g_t[:],
                                                op=ALU.mult)
                        nc.vector.tensor_tensor(s_t[:], g_t[:], m_t[:],
                                                op=ALU.subtract)
                        gru = gp.tile([P, c.CT], FP32, tag="gru")
                        init = 0.0 if batch_start else chain[:, i:i + 1]
                        nc.vector.tensor_tensor_scan(
                            gru[:], a_t[:], s_t[:], init,
                            op0=ALU.mult, op1=ALU.add)
                        nc.vector.tensor_copy(chain[:, i:i + 1],
                                              gru[:, c.CT - 1:c.CT])
                        nc.vector.tensor_tensor(x1r[i][:], gru[:],
                                                hTr[i][:], op=ALU.add)
                        if batch_end:
                            nc.sync.dma_start(
                                GLAST[bidx:bidx + 1, i * P:(i + 1) * P],
                                chain[:, i:i + 1])

                    # ---- FFN1: h1 = gelu(x1 @ W1 + b1) ----
                    h1r = [tp.tile([P, c.CT], F32R, tag=f"h1r{m}", name=f"h1r{m}")
                           for m in range(c.KF)]
                    for m in range(c.KF):
                        if m % 4 == 0:
                            w1s = [tw1.tile([P, 4 * P], F32R, tag=f"w1s{k}", name=f"w1s{k}")
                                   for k in range(c.KD)]
                            for k in range(c.KD):
                                nc.sync.dma_start(
                                    w1s[k][:], W1[k * P:(k + 1) * P,
                                                  m * P:(m + 4) * P])
                        pf = psB.tile([P, c.CT], FP32, tag="pf")
                        off = (m % 4) * P
                        for k in range(c.KD):
                            nc.tensor.matmul(
                                pf[:], w1s[k][:, off:off + P], x1r[k][:],
                                start=(k == 0), stop=(k == c.KD - 1))
                        nc.scalar.activation(h1r[m][:], pf[:], GELU_FUNC,
                                             bias=b1_sb[:, m:m + 1])

                    # ---- FFN2 + residual; x2g staged to DRAM ----
                    x2r = [tp.tile([P, c.CT], F32R, tag=f"x2r{k}", name=f"x2r{k}")
                           for k in range(c.KD)]
                    for mp in range(c.KD // 2):  # m2 pairs
                        pa = psA.tile([P, c.CT], FP32, tag="pf2a")
                        pb = psA.tile([P, c.CT], FP32, tag="pf2b")
                        for k2 in range(c.KF):
                            w2s = tw2.tile([P, 2 * P], F32R,
                                           tag=f"w2s{k2 % 8}")
                            nc.sync.dma_start(
                                w2s[:], W2[k2 * P:(k2 + 1) * P,
                                           mp * 2 * P:(mp + 1) * 2 * P])
                            nc.tensor.matmul(pa[:], w2s[:, 0:P], h1r[k2][:],
                                             start=(k2 == 0),
                                             stop=(k2 == c.KF - 1))
                            nc.tensor.matmul(pb[:], w2s[:, P:2 * P],
                                             h1r[k2][:],
                                             start=(k2 == 0),
                                             stop=(k2 == c.KF - 1))
                        for m2, pf2 in ((2 * mp, pa), (2 * mp + 1, pb)):
                            nc.vector.scalar_tensor_tensor(
                                x2r[m2][:], pf2[:], b2_sb[:, m2:m2 + 1],
                                x1r[m2][:], op0=ALU.add, op1=ALU.add)
                            x2g = tp.tile([P, c.CT], F32R, tag="x2g")
                            nc.vector.tensor_scalar(
                                x2g[:], x2r[m2][:], g2p[:, m2:m2 + 1], None,
                                op0=ALU.mult)
                            nc.sync.dma_start(
                                HN[m2 * P:(m2 + 1) * P,
                                   ch * c.CT:(ch + 1) * c.CT], x2g[:])

                    # ---- rmsnorm2 stats via gram diagonal ----
                    for tb in range(c.TBC):
                        gtb = ch * c.TBC + tb
                        pgr = psA.tile([P, P], FP32, tag="gr")
                        for k in range(c.KD):
                            sl = x2r[k][:, tb * P:(tb + 1) * P]
                            nc.tensor.matmul(pgr[:], sl, sl,
                                             start=(k == 0),
                                             stop=(k == c.KD - 1))
                        dg = tp.tile([P, P], FP32, tag="dg")
                        nc.vector.tensor_tensor(dg[:], pgr[:], ident[:],
                                                op=ALU.mult)
                        ss2 = tp.tile([P, 1], FP32, tag="ss2")
                        nc.vector.tensor_reduce(ss2[:], dg[:],
                                                axis=mybir.AxisListType.X,
                                                op=ALU.add)
                        nc.scalar.activation(ss2[:], ss2[:], AF.Sqrt)
                        nc.vector.tensor_scalar_max(ss2[:], ss2[:], 1e-12)
                        nc.vector.reciprocal(ftok[:, gtb:gtb + 1], ss2[:])

            # =================== logits ===================
            with (
                tc.tile_pool(name="lg", bufs=1) as lp,
                tc.tile_pool(name="lgw", bufs=2) as lw,
                tc.tile_pool(name="lgm", bufs=3) as lm,
                tc.tile_pool(name="psL", bufs=3, space="PSUM") as psL,
            ):
                hNr = [lp.tile([P, c.NT], F32R, tag=f"hNr{k}", name=f"hNr{k}")
                       for k in range(c.KD)]
                for k in range(c.KD):
                    nc.sync.dma_start(hNr[k][:], HN[k * P:(k + 1) * P, :])
                v0 = 0
                for vb, w in enumerate(c.VBW):
                    wv = [lw.tile([P, w], F32R, tag=f"wv{k}", name=f"wv{k}")
                          for k in range(c.KD)]
                    for k in range(c.KD):
                        nc.sync.dma_start(
                            wv[k][:], WLOG[k * P:(k + 1) * P, v0:v0 + w])
                    for tb in range(c.NTB):
                        pl = psL.tile([P, w], FP32, tag="pl")
                        for k in range(c.KD):
                            nc.tensor.matmul(
                                pl[:], hNr[k][:, tb * P:(tb + 1) * P],
                                wv[k][:],
                                start=(k == 0), stop=(k == c.KD - 1))
                        lg_sb = lm.tile([P, w], FP32, tag="lg_sb")
                        nc.vector.tensor_scalar(
                            lg_sb[:], pl[:], ftok[:, tb:tb + 1], SQD,
                            op0=ALU.mult, op1=ALU.mult)
                        nc.sync.dma_start(
                            LOGITS[tb * P:(tb + 1) * P, v0:v0 + w], lg_sb[:])
                        scr = lm.tile([P, w], FP32, tag="scr")
                        se = lm.tile([P, 1], FP32, tag="se")
                        nc.scalar.activation(scr[:], lg_sb[:], AF.Exp,
                                             accum_out=se[:])
                        nc.vector.tensor_tensor(
                            se_acc[:, tb:tb + 1], se_acc[:, tb:tb + 1],
                            se[:], op=ALU.add)
                    v0 += w
                nc.sync.dma_start(SUMEXP[:], se_acc[:])

    nc.compile()
    return nc


def build_program_b(cfg: Cfg):
    """Sequence-parallel trunk (each core owns NT/ncores tokens) +
    vocab-parallel logits. Cross-core scan stitch via a tiny grouped
    AllGather of (H_last, P_last); h_norm^T shards AllGathered per
    128-channel slice so the transfer overlaps the FFN/stats tail."""
    c = cfg
    assert c.CT == c.NT // c.ncores
    G = c.G
    nc = bacc.Bacc("TRN2", target_bir_lowering=False, debug=False,
                   num_devices=c.ncores)

    TOK = nc.dram_tensor("tok", [c.TBC, P], I32, kind="ExternalInput")
    EMB = nc.dram_tensor("emb", [c.V, c.D], FP32, kind="ExternalInput")
    G1 = nc.dram_tensor("g1", [c.KD, P], FP32, kind="ExternalInput")
    G2 = nc.dram_tensor("g2", [c.KD, P], FP32, kind="ExternalInput")
    B1 = nc.dram_tensor("b1", [c.KF, P], FP32, kind="ExternalInput")
    B2 = nc.dram_tensor("b2", [c.KD, P], FP32, kind="ExternalInput")
    WHG = nc.dram_tensor("whg", [c.D, 2 * c.D], F32R, kind="ExternalInput")
    W1 = nc.dram_tensor("w1", [c.D, c.FF], F32R, kind="ExternalInput")
    W2 = nc.dram_tensor("w2", [c.FF, c.D], F32R, kind="ExternalInput")
    WLOG = nc.dram_tensor("wlog", [c.D, c.VSH], F32R, kind="ExternalInput")
    CM = nc.dram_tensor("cmask", [c.ncores, P], FP32, kind="ExternalInput")
    ICM = nc.dram_tensor("icmask", [c.ncores, P], FP32, kind="ExternalInput")

    LOGITS = nc.dram_tensor("logits_s", [c.NT, c.VSH], FP32,
                            kind="ExternalOutput")
    SUMEXP = nc.dram_tensor("sumexp", [P, c.NTB], FP32, kind="ExternalOutput")
    GLAST = nc.dram_tensor("gru_last", [1, c.D], FP32, kind="ExternalOutput")

    world = [list(range(c.ncores))]

    SQD = float(math.sqrt(c.D))

    with tile.TileContext(nc) as tc:
        with (
            tc.tile_pool(name="persist", bufs=1) as pp,
            tc.tile_pool(name="dstage", bufs=1, space="DRAM") as dp,
        ):
            ident = pp.tile([P, P], FP32)
            make_identity(nc, ident[:])

            tok_sb = pp.tile([P, c.TBC], I32)
            for tb in range(c.TBC):
                nc.sync.dma_start(tok_sb[:, tb:tb + 1], TOK[tb:tb + 1, :])

            g1p = pp.tile([P, c.KD], FP32)
            g2p = pp.tile([P, c.KD], FP32)
            b2_sb = pp.tile([P, c.KD], FP32)
            b1_sb = pp.tile([P, c.KF], FP32)
            cm_sb = pp.tile([P, c.ncores], FP32)
            icm_sb = pp.tile([P, c.ncores], FP32)
            for k in range(c.KD):
                nc.sync.dma_start(g1p[:, k:k + 1], G1[k:k + 1, :])
                nc.sync.dma_start(g2p[:, k:k + 1], G2[k:k + 1, :])
                nc.sync.dma_start(b2_sb[:, k:k + 1], B2[k:k + 1, :])
            for m in range(c.KF):
                nc.sync.dma_start(b1_sb[:, m:m + 1], B1[m:m + 1, :])
            for j in range(c.ncores):
                nc.sync.dma_start(cm_sb[:, j:j + 1], CM[j:j + 1, :])
                nc.sync.dma_start(icm_sb[:, j:j + 1], ICM[j:j + 1, :])
            nc.vector.tensor_scalar(g1p[:], g1p[:], 1.0, None, op0=ALU.add)
            nc.vector.tensor_scalar(g2p[:], g2p[:], 1.0, None, op0=ALU.add)

            ftok = pp.tile([P, c.NTB], FP32)
            se_acc = pp.tile([P, c.NTB], FP32)
            nc.vector.memset(se_acc[:], 0.0)
            zeros = pp.tile([P, c.CT], FP32)
            nc.vector.memset(zeros[:], 0.0)

            summ_in = dp.tile([2, c.D], FP32, name="summ_in")
            summ_out = dp.tile([2 * c.ncores, c.D], FP32,
                               addr_space="Shared", name="summ_out")
            f_loc = dp.tile([c.TBC, P], FP32, name="f_loc")
            f_all = dp.tile([c.NTB, P], FP32, addr_space="Shared",
                            name="f_all")
            hn_loc = [dp.tile([P, c.CT], F32R, name=f"hn_loc{k}")
                      for k in range(c.KD)]
            hn_all = [dp.tile([c.ncores * P, c.CT], F32R,
                              addr_space="Shared", name=f"hn_all{k}")
                      for k in range(c.KD)]

            # =================== trunk (own chunk only) ===================
            with (
                tc.tile_pool(name="trunk", bufs=1) as tp,
                tc.tile_pool(name="tw1", bufs=1) as tw1,
                tc.tile_pool(name="tw2", bufs=2) as tw2,
                tc.tile_pool(name="psA", bufs=1, space="PSUM") as psA,
                tc.tile_pool(name="psB", bufs=2, space="PSUM") as psB,
            ):
                # ---- gather + rmsnorm1 + transpose ----
                hTr = [tp.tile([P, c.CT], F32R, tag=f"hTr{k}", name=f"hTr{k}")
                       for k in range(c.KD)]
                for tb in range(c.TBC):
                    h0 = tp.tile([P, c.D], FP32, tag="h0")
                    nc.gpsimd.indirect_dma_start(
                        out=h0[:], out_offset=None,
                        in_=EMB[:],
                        in_offset=bass.IndirectOffsetOnAxis(
                            ap=tok_sb[:, tb:tb + 1], axis=0),
                    )
                    h0n = tp.tile([P, c.D], FP32, tag="h0n")
                    ss = tp.tile([P, 1], FP32, tag="ss")
                    nc.scalar.activation(h0n[:], h0[:], AF.Square,
                                         accum_out=ss[:])
                    nc.scalar.activation(ss[:], ss[:], AF.Sqrt)
                    nc.vector.tensor_scalar_max(ss[:], ss[:], 1e-12)
                    rn = tp.tile([P, 1], FP32, tag="rn")
                    nc.vector.reciprocal(rn[:], ss[:])
                    nc.vector.tensor_scalar(
                        h0n[:], h0[:], rn[:, 0:1], SQD,
                        op0=ALU.mult, op1=ALU.mult)
                    for k in range(c.KD):
                        ptr = psA.tile([P, P], FP32, tag="tr")
                        nc.tensor.transpose(
                            ptr[:], h0n[:, k * P:(k + 1) * P], ident[:])
                        nc.vector.tensor_scalar(
                            hTr[k][:, tb * P:(tb + 1) * P], ptr[:],
                            g1p[:, k:k + 1], None, op0=ALU.mult)

                # ---- minGRU local scans (H with init 0, P cumprod) ----
                Hs = [tp.tile([P, c.CT], F32R, tag=f"Hs{i}", name=f"Hs{i}")
                      for i in range(c.KD)]
                Ps = [tp.tile([P, c.CT], F32R, tag=f"Ps{i}", name=f"Ps{i}")
                      for i in range(c.KD)]
                for i in range(c.KD):
                    whs = [tw1.tile([P, P], F32R, tag=f"whs{k}",
                                    name=f"whs{k}") for k in range(c.KD)]
                    wgs = [tw1.tile([P, P], F32R, tag=f"wgs{k}",
                                    name=f"wgs{k}") for k in range(c.KD)]
                    for k in range(c.KD):
                        nc.sync.dma_start(
                            whs[k][:],
                            WHG[k * P:(k + 1) * P, i * P:(i + 1) * P])
                        nc.sync.dma_start(
                            wgs[k][:],
                            WHG[k * P:(k + 1) * P,
                                c.D + i * P:c.D + (i + 1) * P])
                    ph = psA.tile([P, c.CT], FP32, tag="ph")
                    pg = psA.tile([P, c.CT], FP32, tag="pg")
                    for k in range(c.KD):
                        nc.tensor.matmul(ph[:], whs[k][:], hTr[k][:],
                                         start=(k == 0), stop=(k == c.KD - 1))
                    for k in range(c.KD):
                        nc.tensor.matmul(pg[:], wgs[k][:], hTr[k][:],
                                         start=(k == 0), stop=(k == c.KD - 1))
                    a_t = tp.tile([P, c.CT], FP32, tag="a_t")
                    nc.scalar.activation(a_t[:], pg[:], AF.Sigmoid,
                                         scale=-1.0)
                    s_t = tp.tile([P, c.CT], FP32, tag="s_t")
                    nc.scalar.activation(s_t[:], ph[:], AF.Sigmoid)
                    g_t = tp.tile([P, c.CT], FP32, tag="g_t")
                    nc.vector.tensor_scalar(g_t[:], ph[:], 0.5, None,
                                            op0=ALU.add)
                    nc.vector.tensor_tensor(g_t[:], g_t[:], s_t[:],
                                            op=ALU.subtract)
                    m_t = tp.tile([P, c.CT], FP32, tag="m_t")
                    nc.vector.tensor_scalar(m_t[:], ph[:], 0.0, None,
                                            op0=ALU.is_ge)
                    nc.vector.tensor_tensor(g_t[:], g_t[:], m_t[:],
                                            op=ALU.mult)
                    nc.vector.tensor_tensor(g_t[:], g_t[:], s_t[:],
                                            op=ALU.add)
                    nc.vector.tensor_tensor(m_t[:], a_t[:], g_t[:],
                                            op=ALU.mult)
                    nc.vector.tensor_tensor(s_t[:], g_t[:], m_t[:],
                                            op=ALU.subtract)
                    nc.vector.tensor_tensor_scan(
                        Hs[i][:], a_t[:], s_t[:], 0.0,
                        op0=ALU.mult, op1=ALU.add)
                    nc.vector.tensor_tensor_scan(
                        Ps[i][:], a_t[:], zeros[:], 1.0,
                        op0=ALU.mult, op1=ALU.add)
                    # stage summary row pieces
                    nc.sync.dma_start(summ_in[0:1, i * P:(i + 1) * P],
                                      Hs[i][:, c.CT - 1:c.CT].bitcast(FP32))
                    nc.sync.dma_start(summ_in[1:2, i * P:(i + 1) * P],
                                      Ps[i][:, c.CT - 1:c.CT].bitcast(FP32))

                # ---- stitch scans across cores (grouped AllGather) ----
                nc.gpsimd.collective_compute(
                    "AllGather", ALU.bypass, replica_groups=world,
                    ins=[summ_in[:].opt()], outs=[summ_out[:].opt()])
                hp = tp.tile([P, c.KD], FP32, tag="hp")
                psuf = tp.tile([P, c.KD], FP32, tag="psuf")
                nc.vector.memset(hp[:], 0.0)
                nc.vector.memset(psuf[:], 1.0)
                t1 = tp.tile([P, c.KD], FP32, tag="t1")
                for j in range(c.ncores - 1, -1, -1):
                    Hl = tp.tile([P, c.KD], FP32, tag="Hl")
                    Pl = tp.tile([P, c.KD], FP32, tag="Pl")
                    nc.sync.dma_start(
                        Hl[:], summ_out[2 * j, :].rearrange(
                            "(k p) -> p k", p=P))
                    nc.sync.dma_start(
                        Pl[:], summ_out[2 * j + 1, :].rearrange(
                            "(k p) -> p k", p=P))
                    nc.vector.tensor_tensor(t1[:], Hl[:], psuf[:],
                                            op=ALU.mult)
                    nc.vector.tensor_scalar(t1[:], t1[:], cm_sb[:, j:j + 1],
                                            None, op0=ALU.mult)
                    nc.vector.tensor_tensor(hp[:], hp[:], t1[:], op=ALU.add)
                    nc.vector.tensor_scalar(t1[:], Pl[:], cm_sb[:, j:j + 1],
                                            None, op0=ALU.mult)
                    nc.vector.tensor_scalar(t1[:], t1[:], icm_sb[:, j:j + 1],
                                            None, op0=ALU.add)
                    nc.vector.tensor_tensor(psuf[:], psuf[:], t1[:],
                                            op=ALU.mult)

                # gru_true = H + P*h_prev (into Hs); x1 = gru + hTr (into Ps)
                for i in range(c.KD):
                    nc.vector.scalar_tensor_tensor(
                        Hs[i][:], Ps[i][:], hp[:, i:i + 1], Hs[i][:],
                        op0=ALU.mult, op1=ALU.add)
                    nc.sync.dma_start(GLAST[0:1, i * P:(i + 1) * P],
                                      Hs[i][:, c.CT - 1:c.CT].bitcast(FP32))
                    nc.vector.tensor_tensor(Ps[i][:], Hs[i][:], hTr[i][:],
                                            op=ALU.add)
                x1r = Ps

                # ---- FFN1 ----
                h1r = [tp.tile([P, c.CT], F32R, tag=f"h1r{m}",
                               name=f"h1r{m}") for m in range(c.KF)]
                for m in range(c.KF):
                    if m % 4 == 0:
                        w1s = [tw1.tile([P, 4 * P], F32R, tag=f"w1s{k}",
                                        name=f"w1s{k}") for k in range(c.KD)]
                        for k in range(c.KD):
                            nc.sync.dma_start(
                                w1s[k][:], W1[k * P:(k + 1) * P,
                                              m * P:(m + 4) * P])
                    pf = psB.tile([P, c.CT], FP32, tag="pf")
                    off = (m % 4) * P
                    for k in range(c.KD):
                        nc.tensor.matmul(
                            pf[:], w1s[k][:, off:off + P], x1r[k][:],
                            start=(k == 0), stop=(k == c.KD - 1))
                    nc.scalar.activation(h1r[m][:], pf[:], GELU_FUNC,
                                         bias=b1_sb[:, m:m + 1])

                # ---- FFN2 + residual + gamma2 + per-k AllGather ----
                x2r = [tp.tile([P, c.CT], F32R, tag=f"x2r{k}",
                               name=f"x2r{k}") for k in range(c.KD)]
                for mp in range(c.KD // 2):
                    pa = psA.tile([P, c.CT], FP32, tag="pf2a")
                    pb = psA.tile([P, c.CT], FP32, tag="pf2b")
                    for k2 in range(c.KF):
                        w2s = tw2.tile([P, 2 * P], F32R, tag=f"w2s{k2 % 8}",
                                       name=f"w2s{k2 % 8}")
                        nc.sync.dma_start(
                            w2s[:], W2[k2 * P:(k2 + 1) * P,
                                       mp * 2 * P:(mp + 1) * 2 * P])
                        nc.tensor.matmul(pa[:], w2s[:, 0:P], h1r[k2][:],
                                         start=(k2 == 0),
                                         stop=(k2 == c.KF - 1))
                        nc.tensor.matmul(pb[:], w2s[:, P:2 * P], h1r[k2][:],
                                         start=(k2 == 0),
                                         stop=(k2 == c.KF - 1))
                    for m2, pf2 in ((2 * mp, pa), (2 * mp + 1, pb)):
                        nc.vector.scalar_tensor_tensor(
                            x2r[m2][:], pf2[:], b2_sb[:, m2:m2 + 1],
                            x1r[m2][:], op0=ALU.add, op1=ALU.add)
                        x2g = tp.tile([P, c.CT], F32R, tag="x2g")
                        nc.vector.tensor_scalar(
                            x2g[:], x2r[m2][:], g2p[:, m2:m2 + 1], None,
                            op0=ALU.mult)
                        nc.sync.dma_start(hn_loc[m2][:], x2g[:])
                        nc.gpsimd.collective_compute(
                            "AllGather", ALU.bypass, replica_groups=world,
                            ins=[hn_loc[m2][:].opt()],
                            outs=[hn_all[m2][:].opt()])

                # ---- rmsnorm2 stats (local tokens) + f AllGather ----
                for tb in range(c.TBC):
                    pgr = psA.tile([P, P], FP32, tag="gr")
                    for k in range(c.KD):
                        sl = x2r[k][:, tb * P:(tb + 1) * P]
                        nc.tensor.matmul(pgr[:], sl, sl,
                                         start=(k == 0), stop=(k == c.KD - 1))
                    dg = tp.tile([P, P], FP32, tag="dg")
                    nc.vector.tensor_tensor(dg[:], pgr[:], ident[:],
                                            op=ALU.mult)
                    ss2 = tp.tile([P, 1], FP32, tag="ss2")
                    nc.vector.tensor_reduce(ss2[:], dg[:],
                                            axis=mybir.AxisListType.X,
                                            op=ALU.add)
                    nc.scalar.activation(ss2[:], ss2[:], AF.Sqrt)
                    nc.vector.tensor_scalar_max(ss2[:], ss2[:], 1e-12)
                    fcol = tp.tile([P, 1], FP32, tag="fcol")
                    nc.vector.reciprocal(fcol[:], ss2[:])
                    nc.sync.dma_start(f_loc[tb:tb + 1, :], fcol[:])
                nc.gpsimd.collective_compute(
                    "AllGather", ALU.bypass, replica_groups=world,
                    ins=[f_loc[:].opt()], outs=[f_all[:].opt()])
                nc.sync.dma_start(ftok[:], f_all[:].rearrange("t p -> p t"))

            # =================== logits ===================
            with (
                tc.tile_pool(name="lg", bufs=1) as lp,
                tc.tile_pool(name="lgw", bufs=2) as lw,
                tc.tile_pool(name="lgm", bufs=3) as lm,
                tc.tile_pool(name="psL", bufs=3, space="PSUM") as psL,
            ):
                hNr = [lp.tile([P, c.NT], F32R, tag=f"hNr{k}", name=f"hNr{k}")
                       for k in range(c.KD)]
                for k in range(c.KD):
                    for r in range(c.ncores):
                        nc.sync.dma_start(
                            hNr[k][:, r * c.CT:(r + 1) * c.CT],
                            hn_all[k][r * P:(r + 1) * P, :])
                v0 = 0
                for vb, w in enumerate(c.VBW):
                    wv = [lw.tile([P, w], F32R, tag=f"wv{k}", name=f"wv{k}")
                          for k in range(c.KD)]
                    for k in range(c.KD):
                        nc.sync.dma_start(
                            wv[k][:], WLOG[k * P:(k + 1) * P, v0:v0 + w])
                    for tb in range(c.NTB):
                        pl = psL.tile([P, w], FP32, tag="pl")
                        for k in range(c.KD):
                            nc.tensor.matmul(
                                pl[:], hNr[k][:, tb * P:(tb + 1) * P],
                                wv[k][:],
                                start=(k == 0), stop=(k == c.KD - 1))
                        lg_sb = lm.tile([P, w], FP32, tag="lg_sb")
                        nc.vector.tensor_scalar(
                            lg_sb[:], pl[:], ftok[:, tb:tb + 1], SQD,
                            op0=ALU.mult, op1=ALU.mult)
                        nc.sync.dma_start(
                            LOGITS[tb * P:(tb + 1) * P, v0:v0 + w], lg_sb[:])
                        scr = lm.tile([P, w], FP32, tag="scr")
                        se = lm.tile([P, 1], FP32, tag="se")
                        nc.scalar.activation(scr[:], lg_sb[:], AF.Exp,
                                             accum_out=se[:])
                        nc.vector.tensor_tensor(
                            se_acc[:, tb:tb + 1], se_acc[:, tb:tb + 1],
                            se[:], op=ALU.add)
                    v0 += w
                nc.sync.dma_start(SUMEXP[:], se_acc[:])

    nc.compile()
    return nc


def make_in_maps_b(cfg, x, emb, gamma1, W_hg, W1, b1, W2, b2, gamma2,
                   W_logits):
    c = cfg
    x = np.asarray(x)
    tok_full = np.asarray(x[:, :-1]).reshape(-1).astype(np.int32)
    common = {
        "emb": np.ascontiguousarray(np.asarray(emb, np.float32)),
        "g1": np.asarray(gamma1, np.float32).reshape(c.KD, P),
        "g2": np.asarray(gamma2, np.float32).reshape(c.KD, P),
        "b1": np.asarray(b1, np.float32).reshape(c.KF, P),
        "b2": np.asarray(b2, np.float32).reshape(c.KD, P),
        "whg": np.ascontiguousarray(np.asarray(W_hg, np.float32)),
        "w1": np.ascontiguousarray(np.asarray(W1, np.float32)),
        "w2": np.ascontiguousarray(np.asarray(W2, np.float32)),
    }
    wl = np.asarray(W_logits, np.float32)
    in_maps = []
    for r in range(c.ncores):
        sl = wl[:, r * c.VSH:(r + 1) * c.VSH]
        if sl.shape[1] < c.VSH:
            sl = np.pad(sl, ((0, 0), (0, c.VSH - sl.shape[1])))
        bstart = (r // c.G) * c.G
        cm = np.zeros((c.ncores, P), np.float32)
        cm[bstart:r] = 1.0
        m = dict(common)
        m["wlog"] = np.ascontiguousarray(sl)
        m["tok"] = np.ascontiguousarray(
            tok_full[r * c.CT:(r + 1) * c.CT]).reshape(c.TBC, P)
        m["cmask"] = cm
        m["icmask"] = 1.0 - cm
        in_maps.append(m)
    return in_maps


def assemble_b(cfg, x, results):
    c = cfg
    x = np.asarray(x)
    labels = np.asarray(x[:, 1:]).reshape(-1).astype(np.int64)

    logits = np.empty((c.NT, c.V), np.float32)
    se_total = np.zeros(c.NT, np.float64)
    for r in range(c.ncores):
        w = max(0, min(c.VSH, c.V - r * c.VSH))
        if w > 0:
            logits[:, r * c.VSH:r * c.VSH + w] = results[r]["logits_s"][:, :w]
        se = results[r]["sumexp"].astype(np.float64).T.reshape(-1)
        se_total += se - (c.VSH - w)

    lse = np.log(se_total)
    label_logit = logits[np.arange(c.NT), labels].astype(np.float64)
    loss = np.float32(np.mean(lse - label_logit))

    next_hidden = np.stack(
        [results[(b + 1) * c.G - 1]["gru_last"][0] for b in range(c.B)]
    ).reshape(c.B, 1, c.D).astype(np.float32)
    return loss, logits.reshape(c.B, c.S, c.V), next_hidden


_CACHE = {}


def _get_program(cfg: Cfg, plan="B"):
    key = (plan, cfg.D, cfg.V, cfg.B, cfg.S, cfg.ncores, cfg.CT)
    if key not in _CACHE:
        _CACHE[key] = (build_program_b(cfg) if plan == "B"
                       else build_program(cfg))
    return _CACHE[key]


def make_in_maps(cfg, x, emb, gamma1, W_hg, W1, b1, W2, b2, gamma2, W_logits):
    c = cfg
    x = np.asarray(x)
    tok = np.ascontiguousarray(
        np.asarray(x[:, :-1]).reshape(-1).astype(np.int32)
    ).reshape(c.NTB, P)
    common = {
        "tok": tok,
        "emb": np.ascontiguousarray(np.asarray(emb, np.float32)),
        "g1": np.asarray(gamma1, np.float32).reshape(c.KD, P),
        "g2": np.asarray(gamma2, np.float32).reshape(c.KD, P),
        "b1": np.asarray(b1, np.float32).reshape(c.KF, P),
        "b2": np.asarray(b2, np.float32).reshape(c.KD, P),
        "whg": np.ascontiguousarray(np.asarray(W_hg, np.float32)),
        "w1": np.ascontiguousarray(np.asarray(W1, np.float32)),
        "w2": np.ascontiguousarray(np.asarray(W2, np.float32)),
    }
    wl = np.asarray(W_logits, np.float32)
    in_maps = []
    for r in range(c.ncores):
        sl = wl[:, r * c.VSH:(r + 1) * c.VSH]
        if sl.shape[1] < c.VSH:
            sl = np.pad(sl, ((0, 0), (0, c.VSH - sl.shape[1])))
        m = dict(common)
        m["wlog"] = np.ascontiguousarray(sl)
        in_maps.append(m)
    return in_maps


def assemble(cfg, x, results):
    c = cfg
    x = np.asarray(x)
    labels = np.asarray(x[:, 1:]).reshape(-1).astype(np.int64)

    logits = np.empty((c.NT, c.V), np.float32)
    se_total = np.zeros(c.NT, np.float64)
    for r in range(c.ncores):
        w = min(c.VSH, c.V - r * c.VSH)
        logits[:, r * c.VSH:r * c.VSH + w] = results[r]["logits_s"][:, :w]
        se = results[r]["sumexp"].astype(np.float64)  # [P, NTB]
        se = se.T.reshape(-1)                         # token = tb*P + p
        if w < c.VSH:
            se = se - (c.VSH - w)  # zero-padded cols contribute exp(0)=1
        se_total += se

    lse = np.log(se_total)
    label_logit = logits[np.arange(c.NT), labels].astype(np.float64)
    loss = np.float32(np.mean(lse - label_logit))

    next_hidden = results[0]["gru_last"].reshape(
        c.B, 1, c.D).astype(np.float32)
    return loss, logits.reshape(c.B, c.S, c.V), next_hidden


def kernel(x, emb, gamma1, W_hg, W1, b1, W2, b2, gamma2, W_logits):
    plan = os.environ.get("MINGRU_PLAN", "B")
    cfg = Cfg()
    nc = _get_program(cfg, plan)
    mk = make_in_maps_b if plan == "B" else make_in_maps
    asm = assemble_b if plan == "B" else assemble
    in_maps = mk(cfg, x, emb, gamma1, W_hg, W1, b1, W2, b2,
                 gamma2, W_logits)
    res = run_bass_kernel_spmd(nc, in_maps, list(range(cfg.ncores)))
    return asm(cfg, x, res.results)
